# revision 61
# baseline (speedup 1.0000x reference)
"""Multi-head attention (B=4, S=1024, D=1024, H=16, DH=64) on 8 trn2 cores.

Tensor-parallel over heads: core c owns heads {2c, 2c+1}; each core runs
8 attention units (4 batches x 2 heads).  The core's 128-channel input
slice arrives pre-transposed as x2[b] = [128 ch, S] in bf16 (no ones row;
biases are added during the psum->SBUF copies).

Per batch (head pair j0/j1, block-diagonal weights):
  qk proj   [q_j0|q_j1] = blkdiag(Wq0,Wq1)^T @ x2   (1/sqrt(dh)*log2e folded)
            [k_j0|k_j1] = blkdiag(Wk0,Wk1)^T @ x2
            psum -> qT/kT SBUF bf16 via tensor_scalar(+bias) copies (DVE)
  v proj    per t-block: x2_tb^T @ blkdiag(Wv0,Wv1) -> [t,128] psum;
            plain strided convert-copies -> v_sb[j] with a 66-col block
            layout (col 64 = ones -> Z accumulator, col 65 pad); the v
            bias is deferred to the epilogue
Per unit (b, j), per t-block tb (z = scores * log2e on psum):
  scores    zT[t,s] = kT_j[:,tb128]^T @ qT_j  (bf16, two [128,512] mms
            into a ring of 4 single-bank sc tiles)
  exp       expT = 2^z as bf16, one instruction per sc half:
              ACT  (9 halves/unit): exact exp (scale=ln2)
              DVE  (7 halves/unit): one-pass Schraudolph --
              int16(128*z + B) bit-cast as bf16 IS 2^z*(1 +- 3%);
            the exact half alternates per tb so each output row gets a
            50/50 mix of exact and approximated weights
  pv        out[s,e] accumulated per s-block: expT[:,sblk]^T @ v_tb
            ([128,66] psum x8, Z lands in col 64 via the v ones column;
            the bank is zeroed by start=True on its first sblk group)
  epilogue  ACT/DVE copy pv psum to SBUF, DVE computes 1/Z, Pool (which
            cannot touch psum) multiplies by broadcast 1/Z and adds the
            broadcast v-bias (valid since attn weights sum to 1), then
            one [p, blk, e] strided DMA per unit to DRAM.
PV for t-block tb is emitted after scores of tb+2 (pend FIFO, carried
across unit boundaries) so exp latency hides behind PE work; the next
batch's projections are interleaved as filler chunks.  GPSIMD cannot
access PSUM and TensorScalar/STT have no Pool encoding, so Pool only
runs TensorTensor/Memset on SBUF plus DMA issues.
"""

import numpy as np

D = 1024
H = 16
DH = 64
B = 4
S = 1024
NCORES = 8
HPC = H // NCORES  # heads per core = 2
NT = S // 128      # 8 t/s blocks
EB = 66            # v block stride: 64 e + ones col + pad
SCALE = 1.0 / np.sqrt(DH)
LOG2E = 1.4426950408889634
LN2 = 0.6931471805599453

# Schraudolph: int16(128*z + SH_B) bitcast as bf16 ~= 2^z, max rel err ~3.3%
SH_C = 0.04
SH_A = 128.0
SH_B = 128.0 * (127.0 - SH_C)

# GPSIMD (Pool) cannot access PSUM on real hardware, so everything that
# reads/writes psum runs on ACT/DVE; Pool gets SBUF-only work (divides
# from a copied pv, ones-col memsets, DMA issues).  Exp runs per-half:
# half 0 -> ACT exact exp, half 1 -> DVE Schraudolph, except tb4 half 1
# which also goes to ACT (9 ACT / 7 DVE halves balances the engines).

_CACHE = {}


def _split_sync_waits(nc, limit=1):
    """Walrus in this toolchain rejects instructions carrying more than one
    sync-wait; peel extra waits onto wait-only EventSemaphore ops inserted
    just before, on the same engine queue (engine streams are in-order)."""
    import concourse.mybir as mybir

    n = 0
    for bb in nc.main_func.blocks:
        out = []
        for ins in bb.instructions:
            si = ins.sync_info
            if si is not None and len(si.on_wait) > limit:
                waits = list(si.on_wait)
                for w in waits[:-limit]:
                    ev = mybir.InstEventSemaphore(
                        name=f"WSPLIT-{n}", ins=[], outs=[]
                    )
                    n += 1
                    ev.engine = ins.engine
                    ev.sync_info = mybir.SyncInfo(on_wait=[w], on_update=[])
                    out.append(ev)
                ins.sync_info = mybir.SyncInfo(
                    on_wait=waits[-limit:], on_update=list(si.on_update)
                )
            out.append(ins)
        bb.instructions = out
    return n


def _build_bass(split=True):
    from collections import deque

    import concourse.bass as bass
    import concourse.mybir as mybir
    import concourse.tile as tile

    f32 = mybir.dt.float32
    bf = mybir.dt.bfloat16
    i16 = mybir.dt.int16
    Alu = mybir.AluOpType
    nc = bass.Bass()

    x2_d = nc.declare_dram_parameter("x2", [B, 128, S], bf, isOutput=False)
    wq_d = nc.declare_dram_parameter("wq", [128, 128], bf, isOutput=False)
    wk_d = nc.declare_dram_parameter("wk", [128, 128], bf, isOutput=False)
    wv_d = nc.declare_dram_parameter("wv", [128, 128], bf, isOutput=False)
    bq_d = nc.declare_dram_parameter("bq2", [128, 1], f32, isOutput=False)
    bk_d = nc.declare_dram_parameter("bk2", [128, 1], f32, isOutput=False)
    bv_d = nc.declare_dram_parameter("bv2", [128, HPC * DH], f32, isOutput=False)
    out_d = nc.declare_dram_parameter("out", [B, S, HPC, DH], bf, isOutput=True)

    with tile.TileContext(nc) as tc:
        with (
            tc.tile_pool(name="const", bufs=1) as constp,
            tc.tile_pool(name="sb", bufs=2) as sbp,
            tc.tile_pool(name="expp", bufs=2) as expp,
            tc.tile_pool(name="scps", bufs=2, space="PSUM") as scps,
            tc.tile_pool(name="pvps", bufs=2, space="PSUM") as pvps,
            tc.tile_pool(name="prps", bufs=2, space="PSUM") as prps,
        ):
            wq_sb = constp.tile([128, 128], bf)
            wk_sb = constp.tile([128, 128], bf)
            wv_sb = constp.tile([128, 128], bf)
            bq_sb = constp.tile([128, 1], f32)
            bk_sb = constp.tile([128, 1], f32)
            bv_sb = constp.tile([128, HPC * DH], f32)
            # warm the ACT exp table (1283ns load) off the critical path:
            # the Pool memset is emitted BEFORE the weight-DMA issues so the
            # warm exp runs at ~t=0.3us, not behind 3us of DGE setup
            warm = constp.tile([128, 1], f32)
            warm_in = constp.tile([128, 1], f32)
            nc.gpsimd.memset(warm_in[:], 0.0)
            nc.scalar.activation(
                warm[:], warm_in[:], mybir.ActivationFunctionType.Exp,
                scale=LN2,
            )
            nc.gpsimd.dma_start(wq_sb[:], wq_d[:])
            nc.gpsimd.dma_start(bq_sb[:], bq_d[:])
            nc.gpsimd.dma_start(wk_sb[:], wk_d[:])
            nc.gpsimd.dma_start(bk_sb[:], bk_d[:])
            nc.gpsimd.dma_start(wv_sb[:], wv_d[:])
            nc.gpsimd.dma_start(bv_sb[:], bv_d[:])

            units = [(b, j) for b in range(B) for j in range(HPC)]

            x2s = {}

            def fetch_x2(b):
                if b not in x2s:
                    x2s[b] = sbp.tile([128, S], bf, tag="x2", bufs=2,
                                      name=f"x2_{b}")
                    for half in range(2):
                        hs = slice(half * 512, (half + 1) * 512)
                        nc.sync.dma_start(x2s[b][:, hs], x2_d[b, :, hs])
                return x2s[b]

            qks = {}   # b -> (qT, kT) sbuf bf16 [128, S]
            vsbs = {}  # (b, j) -> v_sb bf16 [128, NT*EB]

            def prep_chunks(b):
                """Projection work for batch b as filler chunks."""
                x2 = fetch_x2(b)
                qT = sbp.tile([128, S], bf, tag="qT", bufs=2, name=f"qT_{b}")
                kT = sbp.tile([128, S], bf, tag="kT", bufs=2, name=f"kT_{b}")
                qks[b] = (qT, kT)
                for j in range(HPC):
                    vsbs[(b, j)] = sbp.tile(
                        [128, NT * EB], bf, tag="v", bufs=3, name=f"v_{b}_{j}"
                    )
                chunks = []

                # qk: mm and bias-copy as SEPARATE chunks so the copy
                # enters the ACT/DVE queue a filler slot after its matmul
                # (the psum write-ack takes ~590ns; emitting them together
                # head-blocks the engine). sh-major order so scores(tb0)
                # waits only on the first two copies.
                for sh in range(2):
                    pairs = []
                    for w_sb, b_sb, dst, eng in (
                        (wq_sb, bq_sb, qT, "act"),
                        (wk_sb, bk_sb, kT, "dve"),
                    ):
                        st = {}

                        def qk_mm(w_sb=w_sb, sh=sh, st=st):
                            ss = slice(sh * 512, (sh + 1) * 512)
                            ps = prps.tile([128, 512], f32, tag="proj",
                                           bufs=2, name="qk_ps")
                            nc.tensor.matmul(ps[:], w_sb[:], x2[:, ss],
                                             start=True, stop=True)
                            st["ps"] = ps

                        def qk_cp(b_sb=b_sb, dst=dst, sh=sh, eng=eng, st=st):
                            ss = slice(sh * 512, (sh + 1) * 512)
                            ps = st["ps"]
                            if eng == "act":
                                nc.scalar.activation(
                                    dst[:, ss], ps[:],
                                    mybir.ActivationFunctionType.Identity,
                                    bias=b_sb[:],
                                )
                            else:
                                nc.vector.tensor_scalar(
                                    dst[:, ss], ps[:], b_sb[:], None, Alu.add
                                )
                        pairs.append((qk_mm, qk_cp))
                    chunks.append(pairs[0][0])
                    chunks.append(pairs[1][0])
                    chunks.append(pairs[0][1])
                    chunks.append(pairs[1][1])

                # ones/pad cols of v tiles (written once per unit tile)
                def vinit():
                    for j in range(HPC):
                        vt = vsbs[(b, j)][:].rearrange(
                            "p (t c) -> p t c", c=EB
                        )
                        nc.gpsimd.memset(vt[:, :, DH:DH + 1], 1.0)
                        nc.gpsimd.memset(vt[:, :, DH + 1:EB], 0.0)
                chunks.append(vinit)

                # v: mms and strided copies as separate chunks (same
                # ack-hiding reason as qk)
                vpairs = []
                for g in range(2):
                    stv = {}

                    def v_mm(g=g, stv=stv):
                        ps = prps.tile([128, 512], f32, tag="proj",
                                       bufs=2, name="v_ps")
                        for q in range(4):
                            tb = g * 4 + q
                            nc.tensor.matmul(
                                ps[:, q * 128:(q + 1) * 128],
                                x2[:, tb * 128:(tb + 1) * 128],
                                wv_sb[:],
                                start=True, stop=True,
                            )
                        stv["ps"] = ps

                    def vc(g=g, stv=stv):
                        ps = stv["ps"]
                        ps3 = ps[:].rearrange("p (t c) -> p t c", c=128)
                        for j in range(HPC):
                            # bv is folded into the final divide (sum of
                            # attn weights is 1), so these are plain
                            # convert-copies, split across ACT and DVE
                            dst = vsbs[(b, j)][
                                :, g * 4 * EB:(g * 4 + 4) * EB
                            ].rearrange("p (t c) -> p t c", c=EB)
                            src = ps3[:, :, j * DH:(j + 1) * DH]
                            if j == 0:
                                nc.scalar.activation(
                                    dst[:, :, :DH], src,
                                    mybir.ActivationFunctionType.Copy,
                                )
                            else:
                                nc.vector.tensor_copy(dst[:, :, :DH], src)
                    vpairs.append((v_mm, vc))
                if True:
                    chunks.append(vpairs[0][0])
                    chunks.append(vpairs[1][0])
                    chunks.append(vpairs[0][1])
                    chunks.append(vpairs[1][1])
                return chunks

            def exp_half(tb, sh, expT, sc_ps):
                # alternate which half is exact so every output row s gets
                # a 50/50 mix of exact and Schraudolph weights; the A,D,D,A
                # period-4 pattern keeps each engine 4 ring slots apart
                dst = expT[:, sh * 512:(sh + 1) * 512]
                if sh == (tb % 2) or tb == 4:
                    nc.scalar.activation(
                        dst, sc_ps[:],
                        mybir.ActivationFunctionType.Exp, scale=LN2,
                    )
                else:
                    nc.vector.tensor_scalar(
                        dst.bitcast(i16), sc_ps[:], SH_A, SH_B,
                        Alu.mult, Alu.add,
                    )

            pend = []  # FIFO of deferred PV t-blocks (depth 2)

            def pv_mms(pvA, pvB, v_sb, tb, expT):
                # 4 accumulation groups share each bank; start=True zeroes
                # the whole 2KB zero-region, so only the FIRST matmul into
                # each bank (sblk 0/4 at tb 0) sets it -- it zeroes the
                # sibling groups' regions before anything lands there.
                for sblk in range(NT):
                    pv = pvA if sblk < 4 else pvB
                    cs = slice((sblk % 4) * EB, (sblk % 4) * EB + EB)
                    nc.tensor.matmul(
                        pv[:, cs],
                        expT[:, sblk * 128:(sblk + 1) * 128],
                        v_sb[:, tb * EB:(tb + 1) * EB],
                        start=(tb == 0 and sblk % 4 == 0),
                        stop=(tb == NT - 1),
                        skip_group_check=True,
                    )

            def scores_pv(b, j, fillers, on_prev_done, last=False):
                """scores -> exp -> PV for unit (b, j); returns pv psums."""
                depth = 1 if last else 2
                qT, kT = qks[b]
                v_sb = vsbs[(b, j)]
                js = slice(j * DH, (j + 1) * DH)
                pvA = pvps.tile([128, 4 * EB], f32, tag="pv", bufs=2,
                                name=f"pvA_{b}_{j}")
                pvB = pvps.tile([128, 4 * EB], f32, tag="pv", bufs=2,
                                name=f"pvB_{b}_{j}")
                for tb in range(NT):
                    expT = expp.tile([128, S], bf, tag="expT", bufs=6,
                                     name="expT")
                    # per-half sc tiles (ring of 4 single-bank buffers) so
                    # sc reuse is gated on exp of tb-2's half, not tb-1's
                    for sh in range(2):
                        sc_ps = scps.tile([128, 512], f32, tag="sc", bufs=4,
                                          name="sc_ps")
                        ss = slice(sh * 512, (sh + 1) * 512)
                        nc.tensor.matmul(
                            sc_ps[:],
                            kT[js, tb * 128:(tb + 1) * 128],
                            qT[js, ss],
                            start=True, stop=True,
                        )
                        exp_half(tb, sh, expT, sc_ps)
                    if len(pend) >= depth:
                        item = pend.pop(0)
                        pv_mms(*item)
                        if (item[0] is not pvA and item[3] == NT - 1
                                and on_prev_done is not None):
                            on_prev_done()
                            on_prev_done = None
                    pend.append((pvA, pvB, v_sb, tb, expT))
                    if fillers:
                        fillers.popleft()()
                return pvA, pvB

            def epilogue(b, j, pvA, pvB, last=False):
                """Drain pv psum, divide by the Z column (+bv), DMA out.

                Steady state: ACT+DVE copy pv to SBUF, Pool (which cannot
                touch psum) does the divides from there.  Final unit: DVE
                stt-divides straight from psum and the two DMA halves go
                out on separate queues, shortening the drain tail."""
                o_sb = sbp.tile([128, NT * DH], bf, tag="o", bufs=2,
                                name=f"o_{b}_{j}")
                bv_j = bv_sb[:, j * DH:(j + 1) * DH]
                if not last:
                    # Pool supports only Memset/TensorTensor(Add,Mult) on
                    # SBUF, so: ACT+DVE drain psum, DVE computes 1/Z once,
                    # Pool multiplies by broadcast invz and adds bv
                    pv_sb = sbp.tile([128, 2 * 4 * EB], f32, tag="pvsb",
                                     bufs=2, name=f"pvsb_{b}_{j}")
                    nc.vector.tensor_copy(pv_sb[:, :4 * EB], pvA[:])
                    nc.vector.tensor_copy(pv_sb[:, 4 * EB:], pvB[:])
                    invz = sbp.tile([128, NT], f32, tag="invz", bufs=2,
                                    name="invz")
                    pv3 = pv_sb[:].rearrange("p (t c) -> p t c", c=EB)
                    nc.vector.reciprocal(
                        invz[:].rearrange("p (z o) -> p z o", o=1),
                        pv3[:, :, DH:DH + 1],
                    )
                    iz_b = invz[:, 0:1]
                    iz_b = bass.AP(iz_b.tensor, iz_b.offset,
                                   [iz_b.ap[0], [1, NT], [0, DH]])
                    bv_b = bass.AP(bv_j.tensor, bv_j.offset,
                                   [bv_j.ap[0], [0, NT], [1, DH]])
                    o3 = o_sb[:].rearrange("p (t e) -> p t e", e=DH)
                    nc.gpsimd.tensor_tensor(
                        o3, pv3[:, :, :DH], iz_b, Alu.mult
                    )
                    nc.gpsimd.tensor_tensor(o3, o3, bv_b, Alu.add)
                    dram = out_d[b, :, j, :].rearrange(
                        "(blk p) e -> p blk e", p=128
                    )
                    sb3 = o_sb[:].rearrange("p (blk e) -> p blk e", e=DH)
                    nc.sync.dma_start(dram, sb3)
                    return
                invz = sbp.tile([128, NT], f32, tag="invz", bufs=2,
                                name="invz_last")
                for g in range(2):
                    pv = pvA if g == 0 else pvB
                    pv3 = pv[:].rearrange("p (t c) -> p t c", c=EB)
                    nc.vector.reciprocal(
                        invz[:, 4 * g:4 * g + 4].rearrange(
                            "p (z o) -> p z o", o=1),
                        pv3[:, :, DH:DH + 1],
                    )
                for sblk in range(4):  # g0 on DVE (stt mult+bv)
                    c0 = sblk * EB
                    nc.vector.scalar_tensor_tensor(
                        o_sb[:, sblk * DH:(sblk + 1) * DH],
                        pvA[:, c0:c0 + DH],
                        invz[:, sblk:sblk + 1],
                        bv_j,
                        Alu.mult,
                        Alu.add,
                    )
                for sblk in range(4, NT):  # g1 on ACT (scale), bv via DVE
                    c0 = (sblk % 4) * EB
                    nc.scalar.activation(
                        o_sb[:, sblk * DH:(sblk + 1) * DH],
                        pvB[:, c0:c0 + DH],
                        mybir.ActivationFunctionType.Copy,
                        scale=invz[:, sblk:sblk + 1],
                    )
                bv_b4 = bass.AP(bv_j.tensor, bv_j.offset,
                                [bv_j.ap[0], [0, 4], [1, DH]])
                o3t = o_sb[:, 4 * DH:].rearrange("p (t e) -> p t e", e=DH)
                nc.vector.tensor_tensor(o3t, o3t, bv_b4, Alu.add)
                for g in range(2):
                    dram = out_d[b, 512 * g:512 * g + 512, j, :].rearrange(
                        "(blk p) e -> p blk e", p=128
                    )
                    sb3 = o_sb[:, 256 * g:256 * g + 256].rearrange(
                        "p (blk e) -> p blk e", e=DH
                    )
                    (nc.sync if g == 0 else nc.scalar).dma_start(dram, sb3)


            fillers = deque()
            fetch_x2(0)
            c0 = prep_chunks(0)
            for c in c0[:-2]:
                c()  # batch 0 projections (through the v mms) run up front
            for c in c0[-2:]:
                fillers.append(c)  # v copies overlap unit 0's first scores
            fetch_x2(1)

            pv_prev = None
            for idx, (b, j) in enumerate(units):
                if j == 0 and b + 1 < B:
                    for c in prep_chunks(b + 1):
                        fillers.append(c)
                    if b + 2 < B:
                        fillers.append(lambda b=b: fetch_x2(b + 2))

                on_prev_done = None
                if pv_prev is not None:
                    prev_unit, prev_ps = units[idx - 1], pv_prev

                    def on_prev_done(prev_unit=prev_unit, prev_ps=prev_ps):
                        epilogue(*prev_unit, *prev_ps)

                pv_prev = scores_pv(b, j, fillers, on_prev_done,
                                    last=(idx == len(units) - 1))
            while pend:
                pv_mms(*pend.pop(0))
            while fillers:
                fillers.popleft()()
            epilogue(*units[-1], *pv_prev, last=True)
    if split:
        _split_sync_waits(nc)
    return nc


def _prep_inputs(sequences, Wq, Wk, Wv, bq, bk, bv):
    """Host-side packing: per-core input maps (all bf16 except biases)."""
    import ml_dtypes

    bf16 = ml_dtypes.bfloat16
    sequences = np.ascontiguousarray(np.asarray(sequences, dtype=np.float32))
    Wq = np.asarray(Wq, np.float32)
    Wk = np.asarray(Wk, np.float32)
    Wv = np.asarray(Wv, np.float32)
    bq = np.asarray(bq, np.float32)
    bk = np.asarray(bk, np.float32)
    bv = np.asarray(bv, np.float32)

    # [B, S, D] -> per-core [B, 128, S] channel slice, transposed
    xT = sequences.transpose(0, 2, 1)  # [B, D, S]

    in_maps = []
    for c in range(NCORES):
        h0 = HPC * c
        x2 = np.ascontiguousarray(
            xT[:, c * 128:(c + 1) * 128, :]
        ).astype(bf16)

        wq2 = np.zeros((128, 128), np.float32)
        wk2 = np.zeros((128, 128), np.float32)
        wv2 = np.zeros((128, 128), np.float32)
        bq2 = np.zeros((128, 1), np.float32)
        bk2 = np.zeros((128, 1), np.float32)
        bv2 = np.zeros((128, HPC * DH), np.float32)
        for j in range(HPC):
            h = h0 + j
            sl = slice(j * DH, (j + 1) * DH)
            wq2[sl, sl] = Wq[h].T * (SCALE * LOG2E)
            wk2[sl, sl] = Wk[h].T
            wv2[sl, sl] = Wv[h].T
            bq2[sl, 0] = bq[h] * (SCALE * LOG2E)
            bk2[sl, 0] = bk[h]
            bv2[:, j * DH:(j + 1) * DH] = bv[h][None, :]
        in_maps.append({
            "x2": x2,
            "wq": wq2.astype(bf16),
            "wk": wk2.astype(bf16),
            "wv": wv2.astype(bf16),
            "bq2": bq2,
            "bk2": bk2,
            "bv2": bv2,
        })
    return in_maps


def get_nc():
    if "nc" not in _CACHE:
        _CACHE["nc"] = _build_bass()
    return _CACHE["nc"]


def kernel(sequences, Wq, Wk, Wv, bq, bk, bv):
    from concourse.bass_utils import run_bass_kernel_spmd

    nc = get_nc()
    in_maps = _prep_inputs(sequences, Wq, Wk, Wv, bq, bk, bv)
    res = run_bass_kernel_spmd(nc, in_maps, list(range(NCORES)))
    full = np.empty((B, S, D), np.float32)
    for c in range(NCORES):
        full[:, :, c * HPC * DH:(c + 1) * HPC * DH] = (
            res.results[c]["out"].reshape(B, S, HPC * DH).astype(np.float32)
        )
    return full


# revision 64
# speedup vs baseline: 1.0016x; 1.0016x over previous
"""Multi-head attention (B=4, S=1024, D=1024, H=16, DH=64) on 8 trn2 cores.

Tensor-parallel over heads: core c owns heads {2c, 2c+1}; each core runs
8 attention units (4 batches x 2 heads).  The core's 128-channel input
slice arrives pre-transposed as x2[b] = [128 ch, S] in bf16 (no ones row;
biases are added during the psum->SBUF copies).

Per batch (head pair j0/j1, block-diagonal weights):
  qk proj   [q_j0|q_j1] = blkdiag(Wq0,Wq1)^T @ x2   (1/sqrt(dh)*log2e folded)
            [k_j0|k_j1] = blkdiag(Wk0,Wk1)^T @ x2
            psum -> qT/kT SBUF bf16 via tensor_scalar(+bias) copies (DVE)
  v proj    per t-block: x2_tb^T @ blkdiag(Wv0,Wv1) -> [t,128] psum;
            plain strided convert-copies -> v_sb[j] with a 66-col block
            layout (col 64 = ones -> Z accumulator, col 65 pad); the v
            bias is deferred to the epilogue
Per unit (b, j), per t-block tb (z = scores * log2e on psum):
  scores    zT[t,s] = kT_j[:,tb128]^T @ qT_j  (bf16, two [128,512] mms
            into a ring of 4 single-bank sc tiles)
  exp       expT = 2^z as bf16, one instruction per sc half:
              ACT  (9 halves/unit): exact exp (scale=ln2)
              DVE  (7 halves/unit): one-pass Schraudolph --
              int16(128*z + B) bit-cast as bf16 IS 2^z*(1 +- 3%);
            the exact half alternates per tb so each output row gets a
            50/50 mix of exact and approximated weights
  pv        out[s,e] accumulated per s-block: expT[:,sblk]^T @ v_tb
            ([128,66] psum x8, Z lands in col 64 via the v ones column;
            the bank is zeroed by start=True on its first sblk group)
  epilogue  ACT/DVE copy pv psum to SBUF, DVE computes 1/Z, Pool (which
            cannot touch psum) multiplies by broadcast 1/Z and adds the
            broadcast v-bias (valid since attn weights sum to 1), then
            one [p, blk, e] strided DMA per unit to DRAM.
PV for t-block tb is emitted after scores of tb+2 (pend FIFO, carried
across unit boundaries) so exp latency hides behind PE work; the next
batch's projections are interleaved as filler chunks.  GPSIMD cannot
access PSUM and TensorScalar/STT have no Pool encoding, so Pool only
runs TensorTensor/Memset on SBUF plus DMA issues.
"""

import numpy as np

D = 1024
H = 16
DH = 64
B = 4
S = 1024
NCORES = 8
HPC = H // NCORES  # heads per core = 2
NT = S // 128      # 8 t/s blocks
EB = 66            # v block stride: 64 e + ones col + pad
SCALE = 1.0 / np.sqrt(DH)
LOG2E = 1.4426950408889634
LN2 = 0.6931471805599453

# Schraudolph: int16(128*z + SH_B) bitcast as bf16 ~= 2^z, max rel err ~3.3%
SH_C = 0.04
SH_A = 128.0
SH_B = 128.0 * (127.0 - SH_C)

# GPSIMD (Pool) cannot access PSUM on real hardware, so everything that
# reads/writes psum runs on ACT/DVE; Pool gets SBUF-only work (divides
# from a copied pv, ones-col memsets, DMA issues).  Exp runs per-half:
# half 0 -> ACT exact exp, half 1 -> DVE Schraudolph, except tb4 half 1
# which also goes to ACT (9 ACT / 7 DVE halves balances the engines).

_CACHE = {}


def _split_sync_waits(nc, limit=1):
    """Walrus in this toolchain rejects instructions carrying more than one
    sync-wait; peel extra waits onto wait-only EventSemaphore ops inserted
    just before, on the same engine queue (engine streams are in-order)."""
    import concourse.mybir as mybir

    n = 0
    for bb in nc.main_func.blocks:
        out = []
        for ins in bb.instructions:
            si = ins.sync_info
            if si is not None and len(si.on_wait) > limit:
                waits = list(si.on_wait)
                for w in waits[:-limit]:
                    ev = mybir.InstEventSemaphore(
                        name=f"WSPLIT-{n}", ins=[], outs=[]
                    )
                    n += 1
                    ev.engine = ins.engine
                    ev.sync_info = mybir.SyncInfo(on_wait=[w], on_update=[])
                    out.append(ev)
                ins.sync_info = mybir.SyncInfo(
                    on_wait=waits[-limit:], on_update=list(si.on_update)
                )
            out.append(ins)
        bb.instructions = out
    return n


def _build_bass(split=True):
    from collections import deque

    import concourse.bass as bass
    import concourse.mybir as mybir
    import concourse.tile as tile

    f32 = mybir.dt.float32
    bf = mybir.dt.bfloat16
    i16 = mybir.dt.int16
    Alu = mybir.AluOpType
    nc = bass.Bass()

    x2_d = nc.declare_dram_parameter("x2", [B, 128, S], bf, isOutput=False)
    wq_d = nc.declare_dram_parameter("wq", [128, 128], bf, isOutput=False)
    wk_d = nc.declare_dram_parameter("wk", [128, 128], bf, isOutput=False)
    wv_d = nc.declare_dram_parameter("wv", [128, 128], bf, isOutput=False)
    bq_d = nc.declare_dram_parameter("bq2", [128, 1], f32, isOutput=False)
    bk_d = nc.declare_dram_parameter("bk2", [128, 1], f32, isOutput=False)
    bv_d = nc.declare_dram_parameter("bv2", [128, HPC * DH], f32, isOutput=False)
    out_d = nc.declare_dram_parameter("out", [B, S, HPC, DH], bf, isOutput=True)

    with tile.TileContext(nc) as tc:
        with (
            tc.tile_pool(name="const", bufs=1) as constp,
            tc.tile_pool(name="sb", bufs=2) as sbp,
            tc.tile_pool(name="expp", bufs=2) as expp,
            tc.tile_pool(name="scps", bufs=2, space="PSUM") as scps,
            tc.tile_pool(name="pvps", bufs=2, space="PSUM") as pvps,
            tc.tile_pool(name="prps", bufs=2, space="PSUM") as prps,
        ):
            wq_sb = constp.tile([128, 128], bf)
            wk_sb = constp.tile([128, 128], bf)
            wv_sb = constp.tile([128, 128], bf)
            bq_sb = constp.tile([128, 1], f32)
            bk_sb = constp.tile([128, 1], f32)
            bv_sb = constp.tile([128, HPC * DH], f32)
            # warm the ACT exp table (1283ns load) off the critical path:
            # the Pool memset is emitted BEFORE the weight-DMA issues so the
            # warm exp runs at ~t=0.3us, not behind 3us of DGE setup
            warm = constp.tile([128, 1], f32)
            warm_in = constp.tile([128, 1], f32)
            nc.gpsimd.memset(warm_in[:], 0.0)
            nc.scalar.activation(
                warm[:], warm_in[:], mybir.ActivationFunctionType.Exp,
                scale=LN2,
            )
            nc.gpsimd.dma_start(wq_sb[:], wq_d[:])
            nc.gpsimd.dma_start(bq_sb[:], bq_d[:])
            nc.gpsimd.dma_start(wk_sb[:], wk_d[:])
            nc.gpsimd.dma_start(bk_sb[:], bk_d[:])
            nc.gpsimd.dma_start(wv_sb[:], wv_d[:])
            nc.gpsimd.dma_start(bv_sb[:], bv_d[:])

            units = [(b, j) for b in range(B) for j in range(HPC)]

            x2s = {}

            def fetch_x2(b):
                if b not in x2s:
                    x2s[b] = sbp.tile([128, S], bf, tag="x2", bufs=2,
                                      name=f"x2_{b}")
                    for half in range(2):
                        hs = slice(half * 512, (half + 1) * 512)
                        nc.sync.dma_start(x2s[b][:, hs], x2_d[b, :, hs])
                return x2s[b]

            qks = {}   # b -> (qT, kT) sbuf bf16 [128, S]
            vsbs = {}  # (b, j) -> v_sb bf16 [128, NT*EB]

            def prep_chunks(b):
                """Projection work for batch b as filler chunks."""
                x2 = fetch_x2(b)
                qT = sbp.tile([128, S], bf, tag="qT", bufs=2, name=f"qT_{b}")
                kT = sbp.tile([128, S], bf, tag="kT", bufs=2, name=f"kT_{b}")
                qks[b] = (qT, kT)
                for j in range(HPC):
                    vsbs[(b, j)] = sbp.tile(
                        [128, NT * EB], bf, tag="v", bufs=3, name=f"v_{b}_{j}"
                    )
                chunks = []

                # qk: mm and bias-copy as SEPARATE chunks so the copy
                # enters the ACT/DVE queue a filler slot after its matmul
                # (the psum write-ack takes ~590ns; emitting them together
                # head-blocks the engine). sh-major order so scores(tb0)
                # waits only on the first two copies.
                for sh in range(2):
                    pairs = []
                    for w_sb, b_sb, dst, eng in (
                        (wq_sb, bq_sb, qT, "act"),
                        (wk_sb, bk_sb, kT, "dve"),
                    ):
                        st = {}

                        def qk_mm(w_sb=w_sb, sh=sh, st=st):
                            ss = slice(sh * 512, (sh + 1) * 512)
                            ps = prps.tile([128, 512], f32, tag="proj",
                                           bufs=2, name="qk_ps")
                            nc.tensor.matmul(ps[:], w_sb[:], x2[:, ss],
                                             start=True, stop=True)
                            st["ps"] = ps

                        def qk_cp(b_sb=b_sb, dst=dst, sh=sh, eng=eng, st=st):
                            ss = slice(sh * 512, (sh + 1) * 512)
                            ps = st["ps"]
                            if eng == "act":
                                nc.scalar.activation(
                                    dst[:, ss], ps[:],
                                    mybir.ActivationFunctionType.Identity,
                                    bias=b_sb[:],
                                )
                            else:
                                nc.vector.tensor_scalar(
                                    dst[:, ss], ps[:], b_sb[:], None, Alu.add
                                )
                        pairs.append((qk_mm, qk_cp))
                    chunks.append(pairs[0][0])
                    chunks.append(pairs[1][0])
                    chunks.append(pairs[0][1])
                    chunks.append(pairs[1][1])

                # ones/pad cols of v tiles (written once per unit tile)
                def vinit():
                    for j in range(HPC):
                        vt = vsbs[(b, j)][:].rearrange(
                            "p (t c) -> p t c", c=EB
                        )
                        nc.gpsimd.memset(vt[:, :, DH:DH + 1], 1.0)
                        nc.gpsimd.memset(vt[:, :, DH + 1:EB], 0.0)
                chunks.append(vinit)

                # v: mms and strided copies as separate chunks (same
                # ack-hiding reason as qk)
                vpairs = []
                for g in range(2):
                    stv = {}

                    def v_mm(g=g, stv=stv):
                        ps = prps.tile([128, 512], f32, tag="proj",
                                       bufs=2, name="v_ps")
                        for q in range(4):
                            tb = g * 4 + q
                            nc.tensor.matmul(
                                ps[:, q * 128:(q + 1) * 128],
                                x2[:, tb * 128:(tb + 1) * 128],
                                wv_sb[:],
                                start=True, stop=True,
                            )
                        stv["ps"] = ps

                    def vc(g=g, stv=stv):
                        ps = stv["ps"]
                        ps3 = ps[:].rearrange("p (t c) -> p t c", c=128)
                        for j in range(HPC):
                            # bv is folded into the final divide (sum of
                            # attn weights is 1), so these are plain
                            # convert-copies, split across ACT and DVE
                            dst = vsbs[(b, j)][
                                :, g * 4 * EB:(g * 4 + 4) * EB
                            ].rearrange("p (t c) -> p t c", c=EB)
                            src = ps3[:, :, j * DH:(j + 1) * DH]
                            if j == 0:
                                nc.scalar.activation(
                                    dst[:, :, :DH], src,
                                    mybir.ActivationFunctionType.Copy,
                                )
                            else:
                                nc.vector.tensor_copy(dst[:, :, :DH], src)
                    vpairs.append((v_mm, vc))
                if True:
                    chunks.append(vpairs[0][0])
                    chunks.append(vpairs[1][0])
                    chunks.append(vpairs[0][1])
                    chunks.append(vpairs[1][1])
                return chunks

            def exp_half(tb, sh, expT, sc_ps):
                # alternate which half is exact so every output row s gets
                # a 50/50 mix of exact and Schraudolph weights; the A,D,D,A
                # period-4 pattern keeps each engine 4 ring slots apart
                dst = expT[:, sh * 512:(sh + 1) * 512]
                if sh == (tb % 2) or tb == 4:
                    nc.scalar.activation(
                        dst, sc_ps[:],
                        mybir.ActivationFunctionType.Exp, scale=LN2,
                    )
                else:
                    nc.vector.tensor_scalar(
                        dst.bitcast(i16), sc_ps[:], SH_A, SH_B,
                        Alu.mult, Alu.add,
                    )

            pend = []  # FIFO of deferred PV t-blocks (depth 2)

            def pv_mms(pvA, pvB, v_sb, tb, expT):
                # 4 accumulation groups share each bank; start=True zeroes
                # the whole 2KB zero-region, so only the FIRST matmul into
                # each bank (sblk 0/4 at tb 0) sets it -- it zeroes the
                # sibling groups' regions before anything lands there.
                for sblk in range(NT):
                    pv = pvA if sblk < 4 else pvB
                    cs = slice((sblk % 4) * EB, (sblk % 4) * EB + EB)
                    nc.tensor.matmul(
                        pv[:, cs],
                        expT[:, sblk * 128:(sblk + 1) * 128],
                        v_sb[:, tb * EB:(tb + 1) * EB],
                        start=(tb == 0 and sblk % 4 == 0),
                        stop=(tb == NT - 1),
                        skip_group_check=True,
                    )

            def scores_pv(b, j, fillers, on_prev_done, last=False):
                """scores -> exp -> PV for unit (b, j); returns pv psums."""
                depth = 1 if last else 2
                qT, kT = qks[b]
                v_sb = vsbs[(b, j)]
                js = slice(j * DH, (j + 1) * DH)
                pvA = pvps.tile([128, 4 * EB], f32, tag="pv", bufs=2,
                                name=f"pvA_{b}_{j}")
                pvB = pvps.tile([128, 4 * EB], f32, tag="pv", bufs=2,
                                name=f"pvB_{b}_{j}")
                for tb in range(NT):
                    expT = expp.tile([128, S], bf, tag="expT", bufs=6,
                                     name="expT")
                    # per-half sc tiles (ring of 4 single-bank buffers) so
                    # sc reuse is gated on exp of tb-2's half, not tb-1's
                    for sh in range(2):
                        sc_ps = scps.tile([128, 512], f32, tag="sc", bufs=4,
                                          name="sc_ps")
                        ss = slice(sh * 512, (sh + 1) * 512)
                        nc.tensor.matmul(
                            sc_ps[:],
                            kT[js, tb * 128:(tb + 1) * 128],
                            qT[js, ss],
                            start=True, stop=True,
                        )
                        exp_half(tb, sh, expT, sc_ps)
                    if len(pend) >= depth:
                        item = pend.pop(0)
                        pv_mms(*item)
                        if (item[0] is not pvA and item[3] == NT - 1
                                and on_prev_done is not None):
                            on_prev_done()
                            on_prev_done = None
                    pend.append((pvA, pvB, v_sb, tb, expT))
                    if fillers:
                        fillers.popleft()()
                return pvA, pvB

            def epilogue(b, j, pvA, pvB, last=False):
                """Drain pv psum, divide by the Z column (+bv), DMA out.

                Steady state: ACT+DVE copy pv to SBUF, Pool (which cannot
                touch psum) does the divides from there.  Final unit: DVE
                stt-divides straight from psum and the two DMA halves go
                out on separate queues, shortening the drain tail."""
                o_sb = sbp.tile([128, NT * DH], bf, tag="o", bufs=2,
                                name=f"o_{b}_{j}")
                bv_j = bv_sb[:, j * DH:(j + 1) * DH]
                if not last:
                    # Pool supports only Memset/TensorTensor(Add,Mult) on
                    # SBUF, so: ACT+DVE drain psum, DVE computes 1/Z once,
                    # Pool multiplies by broadcast invz and adds bv
                    pv_sb = sbp.tile([128, 2 * 4 * EB], f32, tag="pvsb",
                                     bufs=2, name=f"pvsb_{b}_{j}")
                    nc.vector.tensor_copy(pv_sb[:, :4 * EB], pvA[:])
                    nc.vector.tensor_copy(pv_sb[:, 4 * EB:], pvB[:])
                    invz = sbp.tile([128, NT], f32, tag="invz", bufs=2,
                                    name="invz")
                    pv3 = pv_sb[:].rearrange("p (t c) -> p t c", c=EB)
                    nc.vector.reciprocal(
                        invz[:].rearrange("p (z o) -> p z o", o=1),
                        pv3[:, :, DH:DH + 1],
                    )
                    iz_b = invz[:, 0:1]
                    iz_b = bass.AP(iz_b.tensor, iz_b.offset,
                                   [iz_b.ap[0], [1, NT], [0, DH]])
                    bv_b = bass.AP(bv_j.tensor, bv_j.offset,
                                   [bv_j.ap[0], [0, NT], [1, DH]])
                    o3 = o_sb[:].rearrange("p (t e) -> p t e", e=DH)
                    nc.gpsimd.tensor_tensor(
                        o3, pv3[:, :, :DH], iz_b, Alu.mult
                    )
                    nc.gpsimd.tensor_tensor(o3, o3, bv_b, Alu.add)
                    dram = out_d[b, :, j, :].rearrange(
                        "(blk p) e -> p blk e", p=128
                    )
                    sb3 = o_sb[:].rearrange("p (blk e) -> p blk e", e=DH)
                    nc.sync.dma_start(dram, sb3)
                    return
                invz = sbp.tile([128, NT], f32, tag="invz", bufs=2,
                                name="invz_last")
                for g in range(2):
                    pv = pvA if g == 0 else pvB
                    pv3 = pv[:].rearrange("p (t c) -> p t c", c=EB)
                    nc.vector.reciprocal(
                        invz[:, 4 * g:4 * g + 4].rearrange(
                            "p (z o) -> p z o", o=1),
                        pv3[:, :, DH:DH + 1],
                    )
                for sblk in range(4):  # g0 on DVE (stt mult+bv)
                    c0 = sblk * EB
                    nc.vector.scalar_tensor_tensor(
                        o_sb[:, sblk * DH:(sblk + 1) * DH],
                        pvA[:, c0:c0 + DH],
                        invz[:, sblk:sblk + 1],
                        bv_j,
                        Alu.mult,
                        Alu.add,
                    )
                for sblk in range(4, NT):  # g1 on ACT (scale), bv via DVE
                    c0 = (sblk % 4) * EB
                    nc.scalar.activation(
                        o_sb[:, sblk * DH:(sblk + 1) * DH],
                        pvB[:, c0:c0 + DH],
                        mybir.ActivationFunctionType.Copy,
                        scale=invz[:, sblk:sblk + 1],
                    )
                bv_b4 = bass.AP(bv_j.tensor, bv_j.offset,
                                [bv_j.ap[0], [0, 4], [1, DH]])
                o3t = o_sb[:, 4 * DH:].rearrange("p (t e) -> p t e", e=DH)
                nc.vector.tensor_tensor(o3t, o3t, bv_b4, Alu.add)
                for g in range(2):
                    dram = out_d[b, 512 * g:512 * g + 512, j, :].rearrange(
                        "(blk p) e -> p blk e", p=128
                    )
                    sb3 = o_sb[:, 256 * g:256 * g + 256].rearrange(
                        "p (blk e) -> p blk e", e=DH
                    )
                    (nc.sync if g == 0 else nc.scalar).dma_start(dram, sb3)


            fillers = deque()
            fetch_x2(0)
            c0 = prep_chunks(0)
            for c in c0[:-2]:
                c()  # batch 0 projections (through the v mms) run up front
            for c in c0[-2:]:
                fillers.append(c)  # v copies overlap unit 0's first scores
            fetch_x2(1)

            # batch-1's first projection pair runs in the ramp bubble
            # (ACT/DVE are idle until unit 0's first scores land)
            c1 = prep_chunks(1)
            for c in c1[:4]:
                c()
            for c in c1[4:]:
                fillers.append(c)

            pv_prev = None
            for idx, (b, j) in enumerate(units):
                if j == 0 and 2 <= b + 1 < B:
                    for c in prep_chunks(b + 1):
                        fillers.append(c)
                    if b + 2 < B:
                        fillers.append(lambda b=b: fetch_x2(b + 2))

                on_prev_done = None
                if pv_prev is not None:
                    prev_unit, prev_ps = units[idx - 1], pv_prev

                    def on_prev_done(prev_unit=prev_unit, prev_ps=prev_ps):
                        epilogue(*prev_unit, *prev_ps)

                pv_prev = scores_pv(b, j, fillers, on_prev_done,
                                    last=(idx == len(units) - 1))
            while pend:
                pv_mms(*pend.pop(0))
            while fillers:
                fillers.popleft()()
            epilogue(*units[-1], *pv_prev, last=True)
    if split:
        _split_sync_waits(nc)
    return nc


def _prep_inputs(sequences, Wq, Wk, Wv, bq, bk, bv):
    """Host-side packing: per-core input maps (all bf16 except biases)."""
    import ml_dtypes

    bf16 = ml_dtypes.bfloat16
    sequences = np.ascontiguousarray(np.asarray(sequences, dtype=np.float32))
    Wq = np.asarray(Wq, np.float32)
    Wk = np.asarray(Wk, np.float32)
    Wv = np.asarray(Wv, np.float32)
    bq = np.asarray(bq, np.float32)
    bk = np.asarray(bk, np.float32)
    bv = np.asarray(bv, np.float32)

    # [B, S, D] -> per-core [B, 128, S] channel slice, transposed
    xT = sequences.transpose(0, 2, 1)  # [B, D, S]

    in_maps = []
    for c in range(NCORES):
        h0 = HPC * c
        x2 = np.ascontiguousarray(
            xT[:, c * 128:(c + 1) * 128, :]
        ).astype(bf16)

        wq2 = np.zeros((128, 128), np.float32)
        wk2 = np.zeros((128, 128), np.float32)
        wv2 = np.zeros((128, 128), np.float32)
        bq2 = np.zeros((128, 1), np.float32)
        bk2 = np.zeros((128, 1), np.float32)
        bv2 = np.zeros((128, HPC * DH), np.float32)
        for j in range(HPC):
            h = h0 + j
            sl = slice(j * DH, (j + 1) * DH)
            wq2[sl, sl] = Wq[h].T * (SCALE * LOG2E)
            wk2[sl, sl] = Wk[h].T
            wv2[sl, sl] = Wv[h].T
            bq2[sl, 0] = bq[h] * (SCALE * LOG2E)
            bk2[sl, 0] = bk[h]
            bv2[:, j * DH:(j + 1) * DH] = bv[h][None, :]
        in_maps.append({
            "x2": x2,
            "wq": wq2.astype(bf16),
            "wk": wk2.astype(bf16),
            "wv": wv2.astype(bf16),
            "bq2": bq2,
            "bk2": bk2,
            "bv2": bv2,
        })
    return in_maps


def get_nc():
    if "nc" not in _CACHE:
        _CACHE["nc"] = _build_bass()
    return _CACHE["nc"]


def kernel(sequences, Wq, Wk, Wv, bq, bk, bv):
    from concourse.bass_utils import run_bass_kernel_spmd

    nc = get_nc()
    in_maps = _prep_inputs(sequences, Wq, Wk, Wv, bq, bk, bv)
    res = run_bass_kernel_spmd(nc, in_maps, list(range(NCORES)))
    full = np.empty((B, S, D), np.float32)
    for c in range(NCORES):
        full[:, :, c * HPC * DH:(c + 1) * HPC * DH] = (
            res.results[c]["out"].reshape(B, S, HPC * DH).astype(np.float32)
        )
    return full
